# revision 8
# baseline (speedup 1.0000x reference)
"""BiLSTM-CRF loss kernel for 8 Trainium2 NeuronCores (SPMD, Bass/Tile).

Strategy:
  - One SPMD program on 8 cores; behavior diverges only via per-core input
    data (weight slices, base offsets) -- no control flow.
  - LSTM x-projections (the parallel matmuls) are token-sharded across all
    8 cores; results AllGathered.
  - The serial LSTM recurrences run on core 0 (forward) and core 1
    (backward, fed time-reversed data so the same code works); other cores
    run the same instructions on zero weights.
  - Recurrent matmul: stationary fp8e3 (e3m4) weights, unit-major gate
    layout (gates^T = W_hh @ h accumulated in PSUM over 4 k-tiles), h kept
    in fp8, c in fp32. xproj folded into PSUM via an identity matmul.
  - Heads token-sharded; CRF den via exp-domain recursion (u' = E^T u o
    exp(em)) on the PE with renorm every 8 steps; num via host one-hot
    encodings contracted on device. Everything after the last AllGather is
    replicated on all cores; core 0's scalar is returned.

Numerics (validated vs fp32 reference in simulation): rel err ~4e-4.
"""
import numpy as np
import ml_dtypes
import concourse.bass as bass
import concourse.mybir as mybir
import concourse.tile as tile
from concourse import bacc
from concourse.bass_utils import run_bass_kernel_spmd

AF = mybir.ActivationFunctionType
F32 = mybir.dt.float32
BF16 = mybir.dt.bfloat16
F8 = mybir.dt.float8e3
I32 = mybir.dt.int32

NPBF16 = ml_dtypes.bfloat16
NPF8 = ml_dtypes.float8_e3m4

S, B, DIN, DBERT, H, L = 128, 32, 256, 768, 512, 2
D0 = DIN + DBERT
G = 4 * H
KP, KM = 32, 64
KT = KP + KM  # 96
N_CORES = 8
TPC = S // N_CORES  # 16 tokens (time steps) per core per shard
RENORM = 8

_CACHE = {}


# ---------------------------------------------------------------- host packing

def pack_lhsT(w):
    """w [M, K] -> [128, (K/128)*ceil(M/128)*M_tile] stationary tile pack.
    Tile (mc, k) at cols (k*nm + mc)*mt, tile shape [128, mt]."""
    M, K = w.shape
    nk = K // 128
    if M % 128 == 0:
        nm, mt = M // 128, 128
    else:
        assert M <= 128
        nm, mt = 1, M
    A = w.reshape(nm, mt, nk, 128)  # [mc, q, k, p]
    return np.ascontiguousarray(A.transpose(3, 2, 0, 1).reshape(128, nk * nm * mt))


def pack_bias(b):
    """b [G] -> [128, 16] with [q, gc] = b[gc*128+q]."""
    return np.ascontiguousarray(b.reshape(16, 128).T)


# ---------------------------------------------------------------- device build

def _emit_xproj(nc, tc, tag, lhsT_sb, nm, rhs_tiles, nk, bias_sb, bias_off,
                out_dram_dir, psdt=BF16):
    """One direction of an x-projection: out[gc] [128, 512] = sum_k
    lhsT(gc,k).T @ rhs_k, + bias, -> DMA to out_dram_dir[:, :, gc*32:+32].
    rhs_tiles: list of nk APs [128, 16, 32] (or [128, 512]).
    out_dram_dir: AP [16, 128, 512]."""
    with (
        tc.tile_pool(name=f"xps{tag}", bufs=2, space="PSUM") as psp,
        tc.tile_pool(name=f"xpo{tag}", bufs=3) as outp,
    ):
        for gc in range(16):
            ps = psp.tile([128, 512], F32, tag="ps")
            for k in range(nk):
                wcol = (k * nm + gc) * 128
                nc.tensor.matmul(ps[:], lhsT_sb[:, wcol:wcol + 128], rhs_tiles[k],
                                 start=(k == 0), stop=(k == nk - 1),
                                 skip_group_check=True)
            ob = outp.tile([128, 512], psdt, tag="ob")
            nc.scalar.activation(ob[:], ps[:], AF.Identity,
                                 bias=bias_sb[:, bias_off + gc:bias_off + gc + 1])
            dst = out_dram_dir[:, :, gc * 32:gc * 32 + 32].rearrange("t q b -> q t b")
            nc.sync.dma_start(dst, ob[:].rearrange("q (t b) -> q t b", t=16))


def _emit_rec(nc, tc, tag, ident, whh_sb, xp_dram, base_sc, h_dram):
    """Full 128-step LSTM recurrence (one direction's data).
    xp_dram [256, 128, 512] bf16, slab idx = base + (t//16)*32 + t%16.
    h_dram [S, 128, 128] fp8."""
    with (
        tc.tile_pool(name=f"rx{tag}", bufs=3) as xpool,
        tc.tile_pool(name=f"rp{tag}", bufs=2, space="PSUM") as pspool,
        tc.tile_pool(name=f"rs{tag}", bufs=2) as sgpool,
        tc.tile_pool(name=f"rt{tag}", bufs=2) as tmpool,
        tc.tile_pool(name=f"rh{tag}", bufs=3) as hpool,
        tc.tile_pool(name=f"rc{tag}", bufs=1) as stpool,
    ):
        h_prev = stpool.tile([128, 128], F8, tag="h0")
        c_st = stpool.tile([128, 128], F32, tag="c0")
        nc.vector.memset(h_prev[:], 0.0)
        nc.vector.memset(c_st[:], 0.0)

        for t in range(S):
            slab = base_sc + ((t // 16) * 32 + (t % 16))
            xp = xpool.tile([128, 512], BF16, tag="xp")
            src = xp_dram[bass.ds(slab, 1)].rearrange("one p m -> (one p) m")
            for q4 in range(4):
                nc.sync.dma_start(xp[:, 128 * q4:128 * (q4 + 1)],
                                  src[:, 128 * q4:128 * (q4 + 1)])

            ps_if = pspool.tile([128, 256], F32, tag="if")
            ps_g = pspool.tile([128, 128], F32, tag="g")
            ps_o = pspool.tile([128, 128], F32, tag="o")
            nc.tensor.matmul(ps_if[:], ident[:], xp[:, 0:256], start=True,
                             stop=False, skip_group_check=True)
            nc.tensor.matmul(ps_g[:], ident[:], xp[:, 256:384], start=True,
                             stop=False, skip_group_check=True)
            nc.tensor.matmul(ps_o[:], ident[:], xp[:, 384:512], start=True,
                             stop=False, skip_group_check=True)
            for gc in range(16):
                if gc < 8:
                    dst = ps_if[:, 32 * gc:32 * gc + 32]
                elif gc < 12:
                    dst = ps_g[:, 32 * (gc - 8):32 * (gc - 8) + 32]
                else:
                    dst = ps_o[:, 32 * (gc - 12):32 * (gc - 12) + 32]
                for j in range(4):
                    wcol = (j * 16 + gc) * 128
                    nc.tensor.matmul(dst, whh_sb[:, wcol:wcol + 128],
                                     h_prev[:, 32 * j:32 * j + 32],
                                     start=False, stop=(j == 3),
                                     skip_group_check=True)

            s_if = sgpool.tile([128, 256], F32, tag="sif")
            nc.scalar.activation(s_if[:], ps_if[:], AF.Sigmoid)
            s_g = sgpool.tile([128, 128], F32, tag="sg")
            nc.scalar.activation(s_g[:], ps_g[:], AF.Tanh)
            t1 = tmpool.tile([128, 128], F32, tag="t1")
            nc.vector.tensor_mul(t1[:], s_if[:, 0:128], s_g[:])
            nc.vector.tensor_mul(c_st[:], c_st[:], s_if[:, 128:256])
            nc.vector.tensor_add(c_st[:], c_st[:], t1[:])
            s_o = tmpool.tile([128, 128], F32, tag="so")
            nc.scalar.activation(s_o[:], ps_o[:], AF.Sigmoid)
            tc_t = tmpool.tile([128, 128], F32, tag="tc")
            nc.scalar.activation(tc_t[:], c_st[:], AF.Tanh)
            h_new = hpool.tile([128, 128], F8, tag="h")
            nc.vector.tensor_mul(h_new[:], s_o[:], tc_t[:])
            nc.sync.dma_start(h_dram[t], h_new[:])
            h_prev = h_new


def _h_rhs_tiles(nc, tc, tag, h_all, tb_sc, trev_sc, pool):
    """Build 8 rhs k-tiles [128, 16, 32] for tokens t = tbase+dt from
    h_all [1024, 128, 128] (fwd slabs t, bwd slabs 128+(127-t)).
    Returns list of 8 SBUF APs."""
    tiles = []
    for k in range(8):
        rt = pool.tile([128, 512], F8, tag=f"hr{k}")
        col = 32 * (k % 4)
        if k < 4:
            src = h_all[bass.ds(tb_sc, 16), :, col:col + 32]
            nc.sync.dma_start(rt[:].rearrange("p (t b) -> p t b", t=16),
                              src.rearrange("t p b -> p t b"))
        else:
            for dt in range(16):
                src = h_all[bass.ds(trev_sc - dt, 1), :, col:col + 32]
                nc.sync.dma_start(
                    rt[:, 32 * dt:32 * dt + 32],
                    src.rearrange("one p b -> (one p) b"))
        tiles.append(rt[:].rearrange("p (t b) -> p t b", t=16))
    return tiles


def build(n_cores=N_CORES, debug=False):
    nc = bacc.Bacc("TRN2", target_bir_lowering=False, debug=False,
                   num_devices=n_cores)

    # ---------------- inputs (per core)
    srcT_f = nc.dram_tensor("srcT_f", [2, 128, 512], BF16, kind="ExternalInput")
    srcT_b = nc.dram_tensor("srcT_b", [2, 128, 512], BF16, kind="ExternalInput")
    bertT = nc.dram_tensor("bertT", [6, 128, 32], BF16, kind="ExternalInput")
    wih1 = nc.dram_tensor("wih1", [2, 128, 16384], BF16, kind="ExternalInput")
    wih2 = nc.dram_tensor("wih2", [2, 128, 16384], F8, kind="ExternalInput")
    whh1 = nc.dram_tensor("whh1", [128, 8192], F8, kind="ExternalInput")
    whh2 = nc.dram_tensor("whh2", [128, 8192], F8, kind="ExternalInput")
    bias1 = nc.dram_tensor("bias1", [128, 32], F32, kind="ExternalInput")
    bias2 = nc.dram_tensor("bias2", [128, 32], F32, kind="ExternalInput")
    whead = nc.dram_tensor("whead", [128, 768], F8, kind="ExternalInput")
    bhead = nc.dram_tensor("bhead", [96, 1], F32, kind="ExternalInput")
    elog = nc.dram_tensor("elog", [96, 96], F32, kind="ExternalInput")
    startv = nc.dram_tensor("startv", [96, 1], F32, kind="ExternalInput")
    endv = nc.dram_tensor("endv", [96, 1], F32, kind="ExternalInput")
    onesblk = nc.dram_tensor("onesblk", [96, 2], BF16, kind="ExternalInput")
    onesblkT = nc.dram_tensor("onesblkT", [2, 96], BF16, kind="ExternalInput")
    ones96 = nc.dram_tensor("ones96", [96, 1], F32, kind="ExternalInput")
    oh_rows = nc.dram_tensor("oh_rows", [32, 128, 96], BF16, kind="ExternalInput")
    ohn_rows = nc.dram_tensor("ohn_rows", [32, 128, 96], BF16, kind="ExternalInput")
    ohT = nc.dram_tensor("ohT", [96, 4096], BF16, kind="ExternalInput")
    transblk = nc.dram_tensor("transblk", [96, 96], F32, kind="ExternalInput")
    identin = nc.dram_tensor("identin", [128, 128], BF16, kind="ExternalInput")
    cfg = nc.dram_tensor("cfg", [1, 8], I32, kind="ExternalInput")

    loss_out = nc.dram_tensor("loss", [1, 1], F32, kind="ExternalOutput")
    if debug:
        dbg_xp1 = nc.dram_tensor("dbg_xp1", [256, 128, 512], BF16,
                                 kind="ExternalOutput")
        dbg_h1 = nc.dram_tensor("dbg_h1", [1024, 128, 128], F8,
                                kind="ExternalOutput")
        dbg_h2 = nc.dram_tensor("dbg_h2", [1024, 128, 128], F8,
                                kind="ExternalOutput")
        dbg_em = nc.dram_tensor("dbg_em", [768, 512], F32, kind="ExternalOutput")
        dbg_parts = nc.dram_tensor("dbg_parts", [96, 8], F32,
                                   kind="ExternalOutput")

    # ---------------- internal DRAM
    xp1_mine = nc.dram_tensor("xp1_mine", [32, 128, 512], BF16)
    xp1_all = nc.dram_tensor("xp1_all", [256, 128, 512], BF16)
    xp2_mine = nc.dram_tensor("xp2_mine", [32, 128, 512], BF16)
    xp2_all = nc.dram_tensor("xp2_all", [256, 128, 512], BF16)
    h1_mine = nc.dram_tensor("h1_mine", [128, 128, 128], F8)
    h1_all = nc.dram_tensor("h1_all", [1024, 128, 128], F8)
    h2_mine = nc.dram_tensor("h2_mine", [128, 128, 128], F8)
    h2_all = nc.dram_tensor("h2_all", [1024, 128, 128], F8)
    em_mine = nc.dram_tensor("em_mine", [96, 512], F32)
    em_all = nc.dram_tensor("em_all", [768, 512], F32)

    RG = [list(range(n_cores))]

    with tile.TileContext(nc) as tc:
        with tc.tile_pool(name="const", bufs=1) as cpool:
            ident = cpool.tile([128, 128], BF16)
            nc.sync.dma_start(ident[:], identin[:])
            cfg_sb = cpool.tile([1, 8], I32)
            nc.sync.dma_start(cfg_sb[:], cfg[:])

            def ld_reg(idx, mn, mx):
                r = nc.sync.alloc_register(f"cfgr{idx}")
                nc.sync.reg_load(r, cfg_sb[0:1, idx:idx + 1])
                return nc.sync.snap(r, donate=True, min_val=mn, max_val=mx)

            base_sc = ld_reg(0, 0, 16)     # 0 fwd / 16 bwd
            tb_sc = ld_reg(1, 0, 112)      # 16*c
            trev_sc = ld_reg(2, 143, 255)  # 255 - 16*c

            # ============ P0: layer-1 x-projection (token-sharded)
            with tc.tile_pool(name="p0", bufs=1) as p0pool:
                wih1_sb = p0pool.tile([128, 32768], BF16)
                nc.sync.dma_start(wih1_sb[:, 0:16384], wih1[0])
                nc.sync.dma_start(wih1_sb[:, 16384:32768], wih1[1])
                bias1_sb = p0pool.tile([128, 32], F32)
                nc.sync.dma_start(bias1_sb[:], bias1[:])
                bert_sb = p0pool.tile([128, 192], BF16)
                nc.sync.dma_start(bert_sb[:].rearrange("p (m b) -> p m b", m=6),
                                  bertT.ap().rearrange("m p b -> p m b"))
                src_sb = p0pool.tile([128, 2048], BF16)
                nc.sync.dma_start(src_sb[:, 0:1024].rearrange("p (k m) -> p k m", k=2),
                                  srcT_f.ap().rearrange("k p m -> p k m"))
                nc.sync.dma_start(src_sb[:, 1024:2048].rearrange("p (k m) -> p k m", k=2),
                                  srcT_b.ap().rearrange("k p m -> p k m"))

                def bert_tile(m):
                    ap = bert_sb[:, m * 32:m * 32 + 32]
                    return bass.AP(ap.tensor, ap.offset,
                                   [list(ap.ap[0]), [0, 16], [1, 32]])

                for d in range(2):
                    rhs = [src_sb[:, d * 1024 + k * 512:d * 1024 + (k + 1) * 512]
                           .rearrange("p (t b) -> p t b", t=16) for k in range(2)]
                    rhs += [bert_tile(m) for m in range(6)]
                    _emit_xproj(nc, tc, f"x1d{d}", wih1_sb[:, d * 16384:],
                                16, rhs, 8, bias1_sb, d * 16,
                                xp1_mine[d * 16:(d + 1) * 16])

            nc.gpsimd.collective_compute(
                "AllGather", mybir.AluOpType.bypass, replica_groups=RG,
                ins=[xp1_mine.ap().opt()], outs=[xp1_all.ap().opt()])

            # ============ P1: layer-1 recurrence (cores 0/1 carry real data)
            with tc.tile_pool(name="w1", bufs=1) as w1pool:
                whh1_sb = w1pool.tile([128, 8192], F8)
                nc.sync.dma_start(whh1_sb[:], whh1[:])
                _emit_rec(nc, tc, "r1", ident, whh1_sb, xp1_all, base_sc, h1_mine)

            nc.gpsimd.collective_compute(
                "AllGather", mybir.AluOpType.bypass, replica_groups=RG,
                ins=[h1_mine.ap().opt()], outs=[h1_all.ap().opt()])

            # ============ P2: layer-2 x-projection (token-sharded)
            with tc.tile_pool(name="p2", bufs=1) as p2pool:
                wih2_sb = p2pool.tile([128, 32768], F8)
                nc.sync.dma_start(wih2_sb[:, 0:16384], wih2[0])
                nc.sync.dma_start(wih2_sb[:, 16384:32768], wih2[1])
                bias2_sb = p2pool.tile([128, 32], F32)
                nc.sync.dma_start(bias2_sb[:], bias2[:])
                rhs_t = _h_rhs_tiles(nc, tc, "p2h", h1_all, tb_sc, trev_sc, p2pool)
                for d in range(2):
                    _emit_xproj(nc, tc, f"x2d{d}", wih2_sb[:, d * 16384:],
                                16, rhs_t, 8, bias2_sb, d * 16,
                                xp2_mine[d * 16:(d + 1) * 16])

            nc.gpsimd.collective_compute(
                "AllGather", mybir.AluOpType.bypass, replica_groups=RG,
                ins=[xp2_mine.ap().opt()], outs=[xp2_all.ap().opt()])

            # ============ P3: layer-2 recurrence
            with tc.tile_pool(name="w2", bufs=1) as w2pool:
                whh2_sb = w2pool.tile([128, 8192], F8)
                nc.sync.dma_start(whh2_sb[:], whh2[:])
                _emit_rec(nc, tc, "r2", ident, whh2_sb, xp2_all, base_sc, h2_mine)

            nc.gpsimd.collective_compute(
                "AllGather", mybir.AluOpType.bypass, replica_groups=RG,
                ins=[h2_mine.ap().opt()], outs=[h2_all.ap().opt()])

            # ============ P4: emissions (token-sharded)
            with (
                tc.tile_pool(name="p4", bufs=1) as p4pool,
                tc.tile_pool(name="p4ps", bufs=1, space="PSUM") as p4ps,
            ):
                whead_sb = p4pool.tile([128, 768], F8)
                nc.sync.dma_start(whead_sb[:], whead[:])
                bhead_sb = p4pool.tile([96, 1], F32)
                nc.sync.dma_start(bhead_sb[:], bhead[:])
                rhs_t = _h_rhs_tiles(nc, tc, "p4h", h2_all, tb_sc, trev_sc, p4pool)
                em_ps = p4ps.tile([96, 512], F32)
                for k in range(8):
                    nc.tensor.matmul(em_ps[:], whead_sb[:, k * 96:(k + 1) * 96],
                                     rhs_t[k], start=(k == 0), stop=(k == 7),
                                     skip_group_check=True)
                em_sb = p4pool.tile([96, 512], F32)
                nc.scalar.activation(em_sb[:], em_ps[:], AF.Identity,
                                     bias=bhead_sb[:, 0:1])
                nc.sync.dma_start(em_mine[:], em_sb[:])

            nc.gpsimd.collective_compute(
                "AllGather", mybir.AluOpType.bypass, replica_groups=RG,
                ins=[em_mine.ap().opt()], outs=[em_all.ap().opt()])

            if debug:
                nc.sync.dma_start(dbg_xp1[:], xp1_all[:])
                nc.sync.dma_start(dbg_h1[:], h1_all[:])
                nc.sync.dma_start(dbg_h2[:], h2_all[:])
                nc.sync.dma_start(dbg_em[:], em_all[:])

            # ============ P5: CRF + num terms (replicated on all cores)
            _emit_crf(nc, tc, em_all, elog, startv, endv, onesblk, onesblkT,
                      ones96, oh_rows, ohn_rows, ohT, transblk, loss_out,
                      dbg_parts if debug else None)

    nc.compile()
    return nc


def _emit_crf(nc, tc, em_all, elog, startv, endv, onesblk, onesblkT, ones96,
              oh_rows, ohn_rows, ohT, transblk, loss_out, dbg_parts):
    NRE = (S - 1) // RENORM  # 15 renorms at t = 8,16,...,120
    with (
        tc.tile_pool(name="crf", bufs=1) as cp,
        tc.tile_pool(name="crfu", bufs=3) as up,
        tc.tile_pool(name="crfps", bufs=2, space="PSUM") as pp,
        tc.tile_pool(name="crfp1", bufs=1, space="PSUM") as pp1,
        tc.tile_pool(name="crft", bufs=2) as tp,
    ):
        em_sb = cp.tile([96, 4096], F32)
        nc.sync.dma_start(em_sb[:].rearrange("k (r m) -> k r m", r=8),
                          em_all.ap().rearrange("(r k) m -> k r m", r=8))
        eem_sb = cp.tile([96, 4096], F32)
        nc.scalar.activation(eem_sb[:], em_sb[:], AF.Exp)

        E_sb = cp.tile([96, 96], BF16)
        elog_sb = cp.tile([96, 96], F32)
        nc.sync.dma_start(elog_sb[:], elog[:])
        nc.scalar.activation(E_sb[:], elog_sb[:], AF.Exp)
        sv_sb = cp.tile([96, 1], F32)
        nc.sync.dma_start(sv_sb[:], startv[:])
        esv = cp.tile([96, 1], F32)
        nc.scalar.activation(esv[:], sv_sb[:], AF.Exp)
        ev_sb = cp.tile([96, 1], F32)
        nc.sync.dma_start(ev_sb[:], endv[:])
        eev = cp.tile([96, 1], F32)
        nc.scalar.activation(eev[:], ev_sb[:], AF.Exp)
        ob_sb = cp.tile([96, 2], BF16)
        nc.sync.dma_start(ob_sb[:], onesblk[:])
        obT_sb = cp.tile([2, 96], BF16)
        nc.sync.dma_start(obT_sb[:], onesblkT[:])
        o96_sb = cp.tile([96, 1], F32)
        nc.sync.dma_start(o96_sb[:], ones96[:])
        sbuf_sig = cp.tile([2, 32 * NRE], F32)  # sigma log buffer [h, b*NRE+k]

        # u0 = eem[:, t=0] * exp(start)
        u = up.tile([96, 32], BF16, tag="u")
        nc.vector.tensor_scalar_mul(u[:], eem_sb[:, 0:32], esv[:, 0:1])

        nre_done = 0
        for t in range(1, S):
            q_ps = pp.tile([96, 32], F32, tag="q")
            nc.tensor.matmul(q_ps[:], E_sb[:], u[:], start=True, stop=True,
                             skip_group_check=True)
            col = (t // 16) * 512 + (t % 16) * 32
            u2 = up.tile([96, 32], BF16, tag="u")
            nc.vector.tensor_mul(u2[:], q_ps[:], eem_sb[:, col:col + 32])
            u = u2
            if t % RENORM == 0 and t < S - 1:
                sig_ps = pp1.tile([2, 32], F32, tag="sg")
                nc.tensor.matmul(sig_ps[:], ob_sb[:], u[:], start=True,
                                 stop=True, skip_group_check=True)
                k = nre_done
                sap = sbuf_sig[:]
                dst = bass.AP(sap.tensor, sap.offset + k,
                              [list(sap.ap[0]), [NRE, 32]])
                nc.vector.tensor_copy(dst, sig_ps[:])
                r_sb = tp.tile([2, 32], F32, tag="r")
                nc.vector.reciprocal(r_sb[:], sig_ps[:])
                r_bf = tp.tile([2, 32], BF16, tag="rb")
                nc.vector.tensor_copy(r_bf[:], r_sb[:])
                rrep_ps = pp.tile([96, 32], F32, tag="rr")
                nc.tensor.matmul(rrep_ps[:], obT_sb[:], r_bf[:], start=True,
                                 stop=True, skip_group_check=True)
                u3 = up.tile([96, 32], BF16, tag="u")
                nc.vector.tensor_mul(u3[:], u[:], rrep_ps[:])
                u = u3
                nre_done += 1

        # final: uend = u * exp(end); sig_end = colsums; den = ln(sig_end)+sum ln(sig)
        uend = up.tile([96, 32], BF16, tag="u")
        nc.vector.tensor_scalar_mul(uend[:], u[:], eev[:, 0:1])
        sig_ps = pp1.tile([2, 32], F32, tag="sg")
        nc.tensor.matmul(sig_ps[:], ob_sb[:], uend[:], start=True, stop=True,
                         skip_group_check=True)
        den = tp.tile([2, 32], F32, tag="den")
        nc.scalar.activation(den[:], sig_ps[:], AF.Ln)
        lsig = tp.tile([2, 32 * NRE], F32, tag="lsig")
        nc.scalar.activation(lsig[:], sbuf_sig[:], AF.Ln)
        lsum = tp.tile([2, 32], F32, tag="lsum")
        nc.vector.reduce_sum(lsum[:],
                             lsig[:].rearrange("h (b k) -> h b k", k=NRE),
                             axis=mybir.AxisListType.X)
        nc.vector.tensor_add(den[:], den[:], lsum[:])

        # ---- num terms. acc [96, 8] f32: col 0 den(+), 1..5 num(-)
        acc = cp.tile([96, 8], F32)
        nc.vector.memset(acc[:], 0.0)
        dsum = tp.tile([2, 1], F32, tag="dsum")
        nc.vector.reduce_sum(dsum[:], den[:], axis=mybir.AxisListType.X)
        nc.vector.tensor_copy(acc[0:2, 0:1], dsum[:])

        ohT_sb = cp.tile([96, 4096], BF16)
        nc.sync.dma_start(ohT_sb[:], ohT[:])
        # em_sc = sum em o onehot  -> acc col 1
        prod = cp.tile([96, 4096], F32)
        nc.vector.tensor_mul(prod[:], em_sb[:], ohT_sb[:])
        nc.vector.reduce_sum(acc[:, 1:2], prod[:], axis=mybir.AxisListType.X,
                             negate=True)
        # start / end dots -> cols 2, 3
        sdot = tp.tile([96, 32], F32, tag="sd")
        nc.vector.tensor_scalar_mul(sdot[:], ohT_sb[:, 0:32], sv_sb[:, 0:1])
        nc.vector.reduce_sum(acc[:, 2:3], sdot[:], axis=mybir.AxisListType.X,
                             negate=True)
        edot = tp.tile([96, 32], F32, tag="ed")
        nc.vector.tensor_scalar_mul(edot[:], ohT_sb[:, 4064:4096], ev_sb[:, 0:1])
        nc.vector.reduce_sum(acc[:, 3:4], edot[:], axis=mybir.AxisListType.X,
                             negate=True)
        # transition pair counts -> col 4
        oh_sb = cp.tile([128, 3072], BF16)
        nc.sync.dma_start(oh_sb[:].rearrange("p (k m) -> p k m", k=32),
                          oh_rows.ap().rearrange("k p m -> p k m"))
        ohn_sb = cp.tile([128, 3072], BF16)
        nc.sync.dma_start(ohn_sb[:].rearrange("p (k m) -> p k m", k=32),
                          ohn_rows.ap().rearrange("k p m -> p k m"))
        cnt_ps = pp1.tile([96, 96], F32, tag="cnt")
        for kt in range(32):
            nc.tensor.matmul(cnt_ps[0:32, 0:32], oh_sb[:, kt * 96:kt * 96 + 32],
                             ohn_sb[:, kt * 96:kt * 96 + 32],
                             start=(kt == 0), stop=(kt == 31),
                             skip_group_check=True)
            nc.tensor.matmul(cnt_ps[0:64, 32:96], oh_sb[:, kt * 96 + 32:kt * 96 + 96],
                             ohn_sb[:, kt * 96 + 32:kt * 96 + 96],
                             start=(kt == 0), stop=(kt == 31),
                             skip_group_check=True)
        tr_sb = cp.tile([96, 96], F32)
        nc.sync.dma_start(tr_sb[:], transblk[:])
        trc = cp.tile([96, 96], F32)
        nc.vector.tensor_mul(trc[0:32, 0:32], cnt_ps[0:32, 0:32], tr_sb[0:32, 0:32])
        nc.vector.tensor_mul(trc[0:64, 32:96], cnt_ps[0:64, 32:96],
                             tr_sb[0:64, 32:96])
        nc.vector.reduce_sum(acc[0:32, 4:5], trc[0:32, 0:32],
                             axis=mybir.AxisListType.X, negate=True)
        nc.vector.reduce_sum(acc[0:64, 5:6], trc[0:64, 32:96],
                             axis=mybir.AxisListType.X, negate=True)

        if dbg_parts is not None:
            nc.sync.dma_start(dbg_parts[:], acc[:])

        # total = colsum over partitions via ones matmul, then row-sum
        tot_ps = pp1.tile([1, 8], F32, tag="tot")
        nc.tensor.matmul(tot_ps[:], o96_sb[:], acc[:], start=True, stop=True,
                         skip_group_check=True)
        lsc = tp.tile([1, 1], F32, tag="lsc")
        nc.vector.reduce_sum(lsc[:], tot_ps[:], axis=mybir.AxisListType.X)
        nc.sync.dma_start(loss_out[:], lsc[:])


# ---------------------------------------------------------------- host driver

def make_inputs(src_segments, bert_encodings, labels_pos, labels_morph,
                w_ih, w_hh, b_ih, b_hh, head_w_pos, head_b_pos,
                head_w_morph, head_b_morph, start_pos, end_pos, trans_pos,
                start_morph, end_morph, trans_morph):
    """Build the list of 8 per-core input dicts."""
    f32 = np.float32
    src = np.asarray(src_segments, f32)
    bert = np.asarray(bert_encodings, f32)

    wih1_pack = np.stack([pack_lhsT(np.asarray(w_ih[0, d], f32)) for d in range(2)])
    wih2_pack = np.stack([pack_lhsT(np.asarray(w_ih[1, d], f32)) for d in range(2)])
    whh_pack = [[pack_lhsT(np.asarray(w_hh[l, d], f32)) for d in range(2)]
                for l in range(2)]
    bias_pack = [np.stack([pack_bias(np.asarray(b_ih[l, d] + b_hh[l, d], f32))
                           for d in range(2)], axis=0).transpose(1, 0, 2)
                 .reshape(128, 32) for l in range(2)]

    wtilde = np.concatenate([np.asarray(head_w_pos, f32),
                             np.asarray(head_w_morph, f32)], axis=0)  # [96,1024]
    whead_pack = pack_lhsT(wtilde)
    bhead = np.concatenate([np.asarray(head_b_pos, f32),
                            np.asarray(head_b_morph, f32)])[:, None]

    elog = np.full((96, 96), -1e30, f32)
    elog[0:32, 0:32] = np.asarray(trans_pos, f32)
    elog[32:96, 32:96] = np.asarray(trans_morph, f32)
    startv = np.concatenate([np.asarray(start_pos, f32),
                             np.asarray(start_morph, f32)])[:, None]
    endv = np.concatenate([np.asarray(end_pos, f32),
                           np.asarray(end_morph, f32)])[:, None]
    # matches cnt_ps device layout: pos block [0:32,0:32], morph [0:64,32:96]
    transblk = np.zeros((96, 96), f32)
    transblk[0:32, 0:32] = np.asarray(trans_pos, f32)
    transblk[0:64, 32:96] = np.asarray(trans_morph, f32)

    onesblk = np.zeros((96, 2), f32)
    onesblk[0:32, 0] = 1.0
    onesblk[32:96, 1] = 1.0
    onesblkT = np.ascontiguousarray(onesblk.T)
    ones96 = np.ones((96, 1), f32)

    lp = np.asarray(labels_pos)
    lm = np.asarray(labels_morph)
    oh = np.zeros((S * B, 96), f32)  # row = t*B+b
    rows = np.arange(S * B)
    oh[rows, lp.reshape(-1)] = 1.0
    oh[rows, 32 + lm.reshape(-1)] = 1.0
    ohn = np.vstack([oh[B:], np.zeros((B, 96), f32)])
    ohT = np.ascontiguousarray(oh.T)  # [96, 4096] col = t*32+b

    ident = np.eye(128, dtype=f32)

    def bf(x):
        return np.ascontiguousarray(x).astype(NPBF16)

    def f8(x):
        return np.ascontiguousarray(x).astype(NPF8)

    shared = {
        "bertT": bf(bert.T.reshape(6, 128, 32)),
        "wih1": bf(wih1_pack), "wih2": f8(wih2_pack),
        "bias1": bias_pack[0].astype(f32), "bias2": bias_pack[1].astype(f32),
        "whead": f8(whead_pack), "bhead": bhead.astype(f32),
        "elog": elog, "startv": startv.astype(f32), "endv": endv.astype(f32),
        "onesblk": bf(onesblk), "onesblkT": bf(onesblkT),
        "ones96": ones96, "ohT": bf(ohT),
        "oh_rows": bf(oh.reshape(32, 128, 96)),
        "ohn_rows": bf(ohn.reshape(32, 128, 96)),
        "transblk": transblk, "identin": bf(ident),
    }

    zero8 = np.zeros((128, 8192), NPF8)
    in_maps = []
    for c in range(N_CORES):
        t0 = TPC * c
        # fwd slice tokens t0..t0+15; bwd slice steps t0..t0+15 -> tokens 127-t
        sf = src[t0:t0 + TPC]                      # [16, 32, 256]
        sbk = src[S - t0 - TPC:S - t0][::-1]       # tokens 127-t0 ... 112-t0
        def srcpack(x):
            # [16, 32, 256] -> [2, 128, 512]: [k, p, dt*32+b] = x[dt, b, k*128+p]
            return bf(x.transpose(2, 0, 1).reshape(2, 128, TPC * B))
        d = dict(shared)
        d["srcT_f"] = srcpack(sf)
        d["srcT_b"] = srcpack(sbk)
        d["whh1"] = f8(whh_pack[0][0]) if c == 0 else (
            f8(whh_pack[0][1]) if c == 1 else zero8)
        d["whh2"] = f8(whh_pack[1][0]) if c == 0 else (
            f8(whh_pack[1][1]) if c == 1 else zero8)
        d["cfg"] = np.array([[16 if c == 1 else 0, t0, 255 - t0, 0, 0, 0, 0, 0]],
                            np.int32)
        in_maps.append(d)
    return in_maps


def kernel(**inputs):
    key = "nc8"
    if key not in _CACHE:
        _CACHE[key] = build(N_CORES)
    nc = _CACHE[key]
    in_maps = make_inputs(**inputs)
    res = run_bass_kernel_spmd(nc, in_maps, core_ids=list(range(N_CORES)))
    return np.float32(res.results[0]["loss"][0, 0])
